# revision 25
# baseline (speedup 1.0000x reference)
"""DLSA block (clustered sparse attention) Trainium2 kernel.

Full-input contract: kernel(**inputs) takes the complete unsharded tensors,
shards batch-dim across 8 NeuronCores, runs a Bass/Tile kernel per core, and
gathers the full output on host.

Algebraic folds done on host (weight-space only, float64 for accuracy):
  A    = Wq^T @ Wk / sqrt(D)      -> scores S = Xg A Xg^T + (bq Wk/sqrt(D)) Xg^T
  bk drops entirely (adds a per-row constant to scores; softmax-invariant).
  Wvo  = Wo @ Wv                  -> V' = Xp Wvo^T  (V and O projections fused)
  bo2  = bo + Wo @ bv             (bv commutes through attention since rows of
                                   softmax sum to 1; added to V' pre-attention)

Per cluster (S=128 pts, D=32 feats) on device, s-major layouts throughout:
  Xg^T,Xp^T via PE transpose (4 clusters/group, block-diagonal partitions)
  Z'^T[f,s] = A^T Xg^T + c       (4x tile_position-packed K=32 matmuls)
  S^T[t,s]  = Xg Z'^T            (packed)
  P^T       = exp(S^T)           (one ACT op per 4-cluster group)
  V''[t,g]  = Xp Wvo^T + bo2     (packed matmuls + one batched bias-add)
  F[s,g]    = P^T.T @ [V''|1]    (ones col yields softmax denom r in col 32)
  out       = F * (1/r)          (batched strided evac straight into store tile)
"""

import sys

for _p in ("/opt/trn_rl_repo",):
    if _p not in sys.path:
        sys.path.insert(0, _p)

from contextlib import ExitStack

import numpy as np

import concourse.bass as bass
import concourse.tile as tile
from concourse import bacc, mybir
from concourse.bass_utils import run_bass_kernel_spmd

F32 = mybir.dt.float32

B, N, D = 16, 16384, 32
C_TOTAL, S = 128, 128          # clusters per batch, points per cluster
N_CORES = 8
B_LOC = B // N_CORES           # batches per core
ROWS = B_LOC * N               # rows per core
SC_CLUSTERS = 32               # clusters per superchunk (one DMA)
SC_ROWS = SC_CLUSTERS * S      # 4096 rows = 512KB per superchunk
N_SC = ROWS // SC_ROWS         # 8 superchunks per core
G = 4                          # clusters per group (one 128x128 transpose)
GROUPS_PER_SC = SC_CLUSTERS // G


def _build_program():
    nc = bacc.Bacc("TRN2", target_bir_lowering=False, debug=False)

    hg = nc.dram_tensor("hg", [ROWS, D], F32, kind="ExternalInput").ap()
    hp = nc.dram_tensor("hp", [ROWS, D], F32, kind="ExternalInput").ap()
    a_blk = nc.dram_tensor("a_blk", [128, 128], F32, kind="ExternalInput").ap()
    cvec = nc.dram_tensor("cvec", [128, 1], F32, kind="ExternalInput").ap()
    wvo_blk = nc.dram_tensor("wvo_blk", [128, 128], F32, kind="ExternalInput").ap()
    bo2_rep = nc.dram_tensor("bo2_rep", [128, G * D], F32, kind="ExternalInput").ap()
    ident = nc.dram_tensor("ident", [128, 128], F32, kind="ExternalInput").ap()
    out = nc.dram_tensor("out", [ROWS, D], F32, kind="ExternalOutput").ap()

    with tile.TileContext(nc) as tc, ExitStack() as ctx:
        consts = ctx.enter_context(tc.tile_pool(name="consts", bufs=1))
        io_pool = ctx.enter_context(tc.tile_pool(name="io", bufs=2))
        xt_pool = ctx.enter_context(tc.tile_pool(name="xt", bufs=3))
        zsb_pool = ctx.enter_context(tc.tile_pool(name="zsb", bufs=2))
        p_pool = ctx.enter_context(tc.tile_pool(name="p", bufs=2))
        small_pool = ctx.enter_context(tc.tile_pool(name="small", bufs=4))
        v33_pool = ctx.enter_context(tc.tile_pool(name="v33", bufs=1))

        # PSUM: 8 banks total. Row-band-concurrent matmuls must land in
        # distinct banks per band (same-partition same-bank concurrent drains
        # from different sub-array row bands wedge the device).
        ps_tr = ctx.enter_context(tc.tile_pool(name="ps_tr", bufs=1, space="PSUM"))
        ps_z = ctx.enter_context(tc.tile_pool(name="ps_z", bufs=1, space="PSUM"))
        ps_work = ctx.enter_context(tc.tile_pool(name="ps_work", bufs=1, space="PSUM"))
        ps_v = ctx.enter_context(tc.tile_pool(name="ps_v", bufs=1, space="PSUM"))
        ps_f = ctx.enter_context(tc.tile_pool(name="ps_f", bufs=1, space="PSUM"))

        # constants
        a_sb = consts.tile([128, 128], F32, tag="a_sb")
        nc.sync.dma_start(a_sb[:], a_blk)
        cvec_sb = consts.tile([128, 1], F32, tag="cvec_sb")
        nc.sync.dma_start(cvec_sb[:], cvec)
        wvo_sb = consts.tile([128, 128], F32, tag="wvo_sb")
        nc.sync.dma_start(wvo_sb[:], wvo_blk)
        bo2_sb = consts.tile([128, G * D], F32, tag="bo2_sb")
        nc.sync.dma_start(bo2_sb[:], bo2_rep)
        ident_sb = consts.tile([128, 128], F32, tag="ident_sb")
        nc.sync.dma_start(ident_sb[:], ident)

        # v33 ring: [t, (c,33)] with ones in col 32 of each 33-block (written once)
        v33_tiles = []
        for i in range(4):
            t = v33_pool.tile([128, G * 33], F32, tag=f"v33_{i}")
            ones_ap = t[:].rearrange("p (c g) -> p c g", g=33)[:, :, 32:33]
            nc.vector.memset(ones_ap, 1.0)
            v33_tiles.append(t)

        g_global = 0
        for sc in range(N_SC):
            rows = slice(sc * SC_ROWS, (sc + 1) * SC_ROWS)
            # split loads into quarters across both HWDGE rings (sync + ACT)
            # so the first group's compute starts after 1/4 of the load and
            # the two rings move data in parallel.
            hg_sc = io_pool.tile([128, SC_CLUSTERS * D], F32, tag="hg_sc")
            hp_sc = io_pool.tile([128, SC_CLUSTERS * D], F32, tag="hp_sc")
            qc = SC_CLUSTERS // 4
            for q in range(4):
                qrows = slice(sc * SC_ROWS + q * qc * S, sc * SC_ROWS + (q + 1) * qc * S)
                qcols = slice(q * qc * D, (q + 1) * qc * D)
                nc.sync.dma_start(
                    hg_sc[:, qcols].rearrange("p (c d) -> p c d", d=D),
                    hg[qrows, :].rearrange("(c s) d -> s c d", s=S),
                )
                nc.scalar.dma_start(
                    hp_sc[:, qcols].rearrange("p (c d) -> p c d", d=D),
                    hp[qrows, :].rearrange("(c s) d -> s c d", s=S),
                )
            out_sc = io_pool.tile([128, SC_CLUSTERS * D], F32, tag="out_sc")

            if True:
                for g in range(GROUPS_PER_SC):
                    cols = slice(g * G * D, (g + 1) * G * D)

                    # transpose [s,(c,f)] -> [(c,f),s] both inputs, one bank
                    tr_ps = ps_tr.tile([128, 256], F32, tag="tr_ps")
                    nc.tensor.transpose(tr_ps[:, 0:128], hg_sc[:, cols], ident_sb[:])
                    nc.tensor.transpose(
                        tr_ps[:, 128:256], hp_sc[:, cols], ident_sb[:]
                    )
                    xt = xt_pool.tile([128, 256], F32, tag="xt")
                    nc.vector.tensor_copy(xt[:], tr_ps[:])
                    xg = xt[:, 0:128]
                    xp = xt[:, 128:256]

                    # Z'^T[(c,f),s] = blockdiag(A)^T Xg^T (+c at evac)
                    z_ps = ps_z.tile([128, 128], F32, tag="z_ps")
                    nc.tensor.matmul(z_ps[:], a_sb[:], xg)
                    zj = zsb_pool.tile([128, 128], F32, tag="z_sb")
                    nc.scalar.activation(
                        zj[:], z_ps[:], mybir.ActivationFunctionType.Identity,
                        bias=cvec_sb[:],
                    )
                    zj = zj[:]

                    # S^T[t,s] = Xg Z'^T: 4 row-banded matmuls, one PSUM bank
                    # per row band (same-partition same-bank drains wedge).
                    wk = ps_work.tile([128, 2048], F32, tag="wk")
                    for c in range(G):
                        p0 = c * 32
                        nc.tensor.matmul(
                            wk[:, c * 512 : c * 512 + 128],
                            xg[p0 : p0 + 32, :],
                            zj[p0 : p0 + 32, :],
                            tile_position=(p0, 0),
                        )
                    wk_view = wk[:].rearrange("p (c q) -> p c q", q=512)
                    p_sb = p_pool.tile([128, 512], F32, tag="p_sb")
                    nc.scalar.activation(
                        p_sb[:].rearrange("p (c q) -> p c q", q=128),
                        wk_view[:, :, 0:128],
                        mybir.ActivationFunctionType.Exp,
                    )

                    # V'[t,(c,g)] = Xp blockdiag(Wvo^T): one matmul
                    v_ps = ps_v.tile([128, 128], F32, tag="v_ps")
                    nc.tensor.matmul(v_ps[:], xp, wvo_sb[:])
                    # V'' = V' + bo2, strided into the v33 ring (ones col kept)
                    v33 = v33_tiles[g_global % 4]
                    nc.vector.tensor_tensor(
                        v33[:].rearrange("p (c g) -> p c g", g=33)[:, :, 0:32],
                        v_ps[:].rearrange("p (c g) -> p c g", g=D),
                        bo2_sb[:].rearrange("p (c g) -> p c g", g=D),
                        mybir.AluOpType.add,
                    )

                    # F_un[s,(c,33)] = P^T.T @ [V''|1]; col 32 of block = r[s]
                    f_ps = ps_f.tile([128, G * 33], F32, tag="f_ps")
                    for c in range(G):
                        nc.tensor.matmul(
                            f_ps[:, c * 33 : (c + 1) * 33],
                            p_sb[:, c * 128 : (c + 1) * 128],
                            v33[:, c * 33 : (c + 1) * 33],
                            tile_position=(0, 0),
                        )
                    f_view = f_ps[:].rearrange("p (c g) -> p c g", g=33)
                    recip = small_pool.tile([128, G], F32, tag="recip")
                    nc.vector.reciprocal(recip[:, :, None], f_view[:, :, 32:33])
                    nc.vector.tensor_tensor(
                        out_sc[:, cols].rearrange("p (c d) -> p c d", d=D),
                        f_view[:, :, 0:32],
                        recip[:, :, None].to_broadcast([128, G, D]),
                        mybir.AluOpType.mult,
                    )
                    g_global += 1

            for q in range(4):
                qrows = slice(sc * SC_ROWS + q * qc * S, sc * SC_ROWS + (q + 1) * qc * S)
                qcols = slice(q * qc * D, (q + 1) * qc * D)
                eng = nc.sync if q % 2 == 0 else nc.scalar
                eng.dma_start(
                    out[qrows, :].rearrange("(c s) d -> s c d", s=S),
                    out_sc[:, qcols].rearrange("p (c d) -> p c d", d=D),
                )

    nc.compile()
    return nc


_PROGRAM = None


def _get_program():
    global _PROGRAM
    if _PROGRAM is None:
        _PROGRAM = _build_program()
    return _PROGRAM


def _host_fold(Wq, bq, Wk, bk, Wv, bv, Wo, bo):
    Wq64, Wk64 = np.asarray(Wq, np.float64), np.asarray(Wk, np.float64)
    Wv64, Wo64 = np.asarray(Wv, np.float64), np.asarray(Wo, np.float64)
    bq64, bv64, bo64 = (np.asarray(x, np.float64) for x in (bq, bv, bo))
    scale = 1.0 / np.sqrt(np.float64(D))
    A = (Wq64.T @ Wk64) * scale                      # [e, f]
    c = (bq64 @ Wk64) * scale                        # [f]
    WvoT = (Wo64 @ Wv64).T                           # [e, g]
    bo2 = bo64 + Wo64 @ bv64                         # [g]
    a_blk = np.zeros((128, 128), np.float32)
    wvo_blk = np.zeros((128, 128), np.float32)
    for cc in range(G):
        a_blk[cc * D : (cc + 1) * D, cc * D : (cc + 1) * D] = A
        wvo_blk[cc * D : (cc + 1) * D, cc * D : (cc + 1) * D] = WvoT
    cvec = np.tile(c, G)[:, None].astype(np.float32)         # [128, 1]
    bo2_rep = np.tile(bo2, (128, G)).reshape(128, G * D).astype(np.float32)
    return a_blk, cvec, wvo_blk, bo2_rep


def make_in_maps(h_pos, h_geo, Wq, bq, Wk, bk, Wv, bv, Wo, bo):
    a_blk, cvec, wvo_blk, bo2_rep = _host_fold(Wq, bq, Wk, bk, Wv, bv, Wo, bo)
    ident = np.eye(128, dtype=np.float32)
    hg_full = np.ascontiguousarray(np.asarray(h_geo, np.float32)).reshape(B * N, D)
    hp_full = np.ascontiguousarray(np.asarray(h_pos, np.float32)).reshape(B * N, D)
    in_maps = []
    for core in range(N_CORES):
        rows = slice(core * ROWS, (core + 1) * ROWS)
        in_maps.append(
            {
                "hg": np.ascontiguousarray(hg_full[rows]),
                "hp": np.ascontiguousarray(hp_full[rows]),
                "a_blk": a_blk,
                "cvec": cvec,
                "wvo_blk": wvo_blk,
                "bo2_rep": bo2_rep,
                "ident": ident,
            }
        )
    return in_maps


def kernel(h_pos, h_geo, n_clusters, Wq, bq, Wk, bk, Wv, bv, Wo, bo, **kwargs):
    assert int(n_clusters) == C_TOTAL
    nc = _get_program()
    in_maps = make_in_maps(h_pos, h_geo, Wq, bq, Wk, bk, Wv, bv, Wo, bo)
    res = run_bass_kernel_spmd(nc, in_maps, core_ids=list(range(N_CORES)))
    shards = [r["out"].reshape(B_LOC, N, D) for r in res.results]
    return np.concatenate(shards, axis=0).astype(np.float32)


# revision 26
# speedup vs baseline: 1.1882x; 1.1882x over previous
"""DLSA block (clustered sparse attention) Trainium2 kernel.

Full-input contract: kernel(**inputs) takes the complete unsharded tensors,
shards batch-dim across 8 NeuronCores, runs a Bass/Tile kernel per core, and
gathers the full output on host.

Host-side marshaling: h_geo/h_pos are uploaded pre-transposed per cluster
([B, C, D, S] layout) so the kernel needs no on-chip transposes and DMA
descriptors are 512B (cluster-feature rows) instead of 128B point rows.

Algebraic folds done on host (weight-space only, float64 for accuracy):
  A    = Wq^T @ Wk / sqrt(D)      -> scores S = Xg A Xg^T + (bq Wk/sqrt(D)) Xg^T
  bk drops entirely (adds a per-row constant to scores; softmax-invariant).
  Wvo  = Wo @ Wv                  -> V' = Xp Wvo^T  (V and O projections fused)
  bo2  = bo + Wo @ bv             (bv commutes through attention since rows of
                                   softmax sum to 1; added to V' pre-attention)

Per cluster (S=128 pts, D=32 feats) on device:
  Z'^T[f,s] = blockdiag(A)^T Xg^T + c   (one matmul per 4-cluster group)
  S^T[t,s]  = Xg Z'^T             (4 row-banded matmuls, one PSUM bank/band)
  P^T       = exp(S^T)            (one ACT op per group)
  V''[t,g]  = Xp blockdiag(Wvo)^T + bo2 (one matmul + one batched bias-add)
  F[s,g]    = P^T.T @ [V''|1]     (ones col yields softmax denom r in col 32)
  out       = F * (1/r)           (batched strided evac into the store tile)
"""

import sys

for _p in ("/opt/trn_rl_repo",):
    if _p not in sys.path:
        sys.path.insert(0, _p)

from contextlib import ExitStack

import numpy as np

import concourse.bass as bass
import concourse.tile as tile
from concourse import bacc, mybir
from concourse.bass_utils import run_bass_kernel_spmd

F32 = mybir.dt.float32

B, N, D = 16, 16384, 32
C_TOTAL, S = 128, 128          # clusters per batch, points per cluster
N_CORES = 8
B_LOC = B // N_CORES           # batches per core
ROWS = B_LOC * N               # data rows per core
TROWS = B_LOC * C_TOTAL * D    # rows of the transposed layout [(b,c,f), s]
SC_CLUSTERS = 32               # clusters per superchunk
SC_ROWS = SC_CLUSTERS * S      # output rows per superchunk
SC_TROWS = SC_CLUSTERS * D     # transposed rows per superchunk
N_SC = ROWS // SC_ROWS         # 8 superchunks per core
G = 4                          # clusters per group
GROUPS_PER_SC = SC_CLUSTERS // G


def _build_program():
    nc = bacc.Bacc("TRN2", target_bir_lowering=False, debug=False)

    hgT = nc.dram_tensor("hgT", [TROWS, S], F32, kind="ExternalInput").ap()
    hpT = nc.dram_tensor("hpT", [TROWS, S], F32, kind="ExternalInput").ap()
    a_blk = nc.dram_tensor("a_blk", [128, 128], F32, kind="ExternalInput").ap()
    cvec = nc.dram_tensor("cvec", [128, 1], F32, kind="ExternalInput").ap()
    wvo_blk = nc.dram_tensor("wvo_blk", [128, 128], F32, kind="ExternalInput").ap()
    bo2_rep = nc.dram_tensor("bo2_rep", [128, G * D], F32, kind="ExternalInput").ap()
    out = nc.dram_tensor("out", [ROWS, D], F32, kind="ExternalOutput").ap()

    with tile.TileContext(nc) as tc, ExitStack() as ctx:
        consts = ctx.enter_context(tc.tile_pool(name="consts", bufs=1))
        io_pool = ctx.enter_context(tc.tile_pool(name="io", bufs=2))
        zsb_pool = ctx.enter_context(tc.tile_pool(name="zsb", bufs=2))
        p_pool = ctx.enter_context(tc.tile_pool(name="p", bufs=2))
        small_pool = ctx.enter_context(tc.tile_pool(name="small", bufs=4))
        v33_pool = ctx.enter_context(tc.tile_pool(name="v33", bufs=1))

        # PSUM: 8 banks. Row-band-concurrent matmuls must land in distinct
        # banks per band (same-partition same-bank concurrent drains from
        # different sub-array row bands wedge the device).
        ps_z = ctx.enter_context(tc.tile_pool(name="ps_z", bufs=1, space="PSUM"))
        ps_work = ctx.enter_context(tc.tile_pool(name="ps_work", bufs=1, space="PSUM"))
        ps_v = ctx.enter_context(tc.tile_pool(name="ps_v", bufs=1, space="PSUM"))
        ps_f = ctx.enter_context(tc.tile_pool(name="ps_f", bufs=2, space="PSUM"))

        # constants
        a_sb = consts.tile([128, 128], F32, tag="a_sb")
        nc.sync.dma_start(a_sb[:], a_blk)
        cvec_sb = consts.tile([128, 1], F32, tag="cvec_sb")
        nc.sync.dma_start(cvec_sb[:], cvec)
        wvo_sb = consts.tile([128, 128], F32, tag="wvo_sb")
        nc.sync.dma_start(wvo_sb[:], wvo_blk)
        bo2_sb = consts.tile([128, G * D], F32, tag="bo2_sb")
        nc.sync.dma_start(bo2_sb[:], bo2_rep)

        # v33 ring: [t, (c,33)] with ones in col 32 of each 33-block
        v33_tiles = []
        for i in range(4):
            t = v33_pool.tile([128, G * 33], F32, tag=f"v33_{i}")
            ones_ap = t[:].rearrange("p (c g) -> p c g", g=33)[:, :, 32:33]
            nc.vector.memset(ones_ap, 1.0)
            v33_tiles.append(t)

        g_global = 0
        for sc in range(N_SC):
            rows = slice(sc * SC_ROWS, (sc + 1) * SC_ROWS)
            trow0 = sc * SC_TROWS
            # hgT/hpT superchunk: [(c4,f)=128, (j, s)] — group j's block-diag
            # transposed inputs land directly in matmul-operand layout.
            # Loads split in half so group 0 can start early.
            hg_sc = io_pool.tile([128, GROUPS_PER_SC * S], F32, tag="hg_sc")
            hp_sc = io_pool.tile([128, GROUPS_PER_SC * S], F32, tag="hp_sc")
            half_j = GROUPS_PER_SC // 2
            for h in range(2):
                r0 = trow0 + h * half_j * 128
                jcols = slice(h * half_j * S, (h + 1) * half_j * S)
                nc.sync.dma_start(
                    hg_sc[:, jcols].rearrange("p (j s) -> p j s", j=half_j),
                    hgT[r0 : r0 + half_j * 128, :].rearrange(
                        "(j r) s -> r j s", j=half_j
                    ),
                )
                nc.sync.dma_start(
                    hp_sc[:, jcols].rearrange("p (j s) -> p j s", j=half_j),
                    hpT[r0 : r0 + half_j * 128, :].rearrange(
                        "(j r) s -> r j s", j=half_j
                    ),
                )
            out_sc = io_pool.tile([128, SC_CLUSTERS * D], F32, tag="out_sc")

            for j in range(GROUPS_PER_SC):
                cols = slice(j * G * D, (j + 1) * G * D)
                xg = hg_sc[:, j * S : (j + 1) * S]
                xp = hp_sc[:, j * S : (j + 1) * S]

                # Z'^T[(c,f),s] = blockdiag(A)^T Xg^T (+c at evac)
                z_ps = ps_z.tile([128, 128], F32, tag="z_ps")
                nc.tensor.matmul(z_ps[:], a_sb[:], xg)
                z_sb = zsb_pool.tile([128, 128], F32, tag="z_sb")
                nc.scalar.activation(
                    z_sb[:], z_ps[:], mybir.ActivationFunctionType.Identity,
                    bias=cvec_sb[:],
                )

                # S^T[t,s] = Xg Z'^T: 4 row-banded matmuls, one bank per band
                wk = ps_work.tile([128, 2048], F32, tag="wk")
                for c in range(G):
                    p0 = c * 32
                    nc.tensor.matmul(
                        wk[:, c * 512 : c * 512 + 128],
                        xg[p0 : p0 + 32, :],
                        z_sb[p0 : p0 + 32, :],
                        tile_position=(p0, 0),
                    )
                wk_view = wk[:].rearrange("p (c q) -> p c q", q=512)
                p_sb = p_pool.tile([128, 512], F32, tag="p_sb")
                nc.scalar.activation(
                    p_sb[:].rearrange("p (c q) -> p c q", q=128),
                    wk_view[:, :, 0:128],
                    mybir.ActivationFunctionType.Exp,
                )

                # V'[t,(c,g)] = Xp blockdiag(Wvo^T): one matmul
                v_ps = ps_v.tile([128, 128], F32, tag="v_ps")
                nc.tensor.matmul(v_ps[:], xp, wvo_sb[:])
                # V'' = V' + bo2, strided into the v33 ring (ones col kept)
                v33 = v33_tiles[g_global % 4]
                nc.vector.tensor_tensor(
                    v33[:].rearrange("p (c g) -> p c g", g=33)[:, :, 0:32],
                    v_ps[:].rearrange("p (c g) -> p c g", g=D),
                    bo2_sb[:].rearrange("p (c g) -> p c g", g=D),
                    mybir.AluOpType.add,
                )

                # F_un[s,(c,33)] = P^T.T @ [V''|1]; col 32 of block = r[s]
                f_ps = ps_f.tile([128, G * 33], F32, tag="f_ps")
                for c in range(G):
                    nc.tensor.matmul(
                        f_ps[:, c * 33 : (c + 1) * 33],
                        p_sb[:, c * 128 : (c + 1) * 128],
                        v33[:, c * 33 : (c + 1) * 33],
                        tile_position=(0, 0),
                    )
                f_view = f_ps[:].rearrange("p (c g) -> p c g", g=33)
                recip = small_pool.tile([128, G], F32, tag="recip")
                nc.vector.reciprocal(recip[:, :, None], f_view[:, :, 32:33])
                nc.vector.tensor_tensor(
                    out_sc[:, cols].rearrange("p (c d) -> p c d", d=D),
                    f_view[:, :, 0:32],
                    recip[:, :, None].to_broadcast([128, G, D]),
                    mybir.AluOpType.mult,
                )
                g_global += 1

            nc.sync.dma_start(
                out[rows, :].rearrange("(c s) d -> s c d", s=S),
                out_sc[:].rearrange("p (c d) -> p c d", d=D),
            )

    nc.compile()
    return nc


_PROGRAM = None


def _get_program():
    global _PROGRAM
    if _PROGRAM is None:
        _PROGRAM = _build_program()
    return _PROGRAM


def _host_fold(Wq, bq, Wk, bk, Wv, bv, Wo, bo):
    Wq64, Wk64 = np.asarray(Wq, np.float64), np.asarray(Wk, np.float64)
    Wv64, Wo64 = np.asarray(Wv, np.float64), np.asarray(Wo, np.float64)
    bq64, bv64, bo64 = (np.asarray(x, np.float64) for x in (bq, bv, bo))
    scale = 1.0 / np.sqrt(np.float64(D))
    A = (Wq64.T @ Wk64) * scale                      # [e, f]
    c = (bq64 @ Wk64) * scale                        # [f]
    WvoT = (Wo64 @ Wv64).T                           # [e, g]
    bo2 = bo64 + Wo64 @ bv64                         # [g]
    a_blk = np.zeros((128, 128), np.float32)
    wvo_blk = np.zeros((128, 128), np.float32)
    for cc in range(G):
        a_blk[cc * D : (cc + 1) * D, cc * D : (cc + 1) * D] = A
        wvo_blk[cc * D : (cc + 1) * D, cc * D : (cc + 1) * D] = WvoT
    cvec = np.tile(c, G)[:, None].astype(np.float32)         # [128, 1]
    bo2_rep = np.tile(bo2, (128, G)).reshape(128, G * D).astype(np.float32)
    return a_blk, cvec, wvo_blk, bo2_rep


def make_in_maps(h_pos, h_geo, Wq, bq, Wk, bk, Wv, bv, Wo, bo):
    a_blk, cvec, wvo_blk, bo2_rep = _host_fold(Wq, bq, Wk, bk, Wv, bv, Wo, bo)
    # per-cluster transpose on host: [B, N, D] -> [B, C, D, S]
    hgT_full = np.ascontiguousarray(
        np.asarray(h_geo, np.float32).reshape(B, C_TOTAL, S, D).transpose(0, 1, 3, 2)
    ).reshape(B * C_TOTAL * D, S)
    hpT_full = np.ascontiguousarray(
        np.asarray(h_pos, np.float32).reshape(B, C_TOTAL, S, D).transpose(0, 1, 3, 2)
    ).reshape(B * C_TOTAL * D, S)
    in_maps = []
    for core in range(N_CORES):
        trows = slice(core * TROWS, (core + 1) * TROWS)
        in_maps.append(
            {
                "hgT": np.ascontiguousarray(hgT_full[trows]),
                "hpT": np.ascontiguousarray(hpT_full[trows]),
                "a_blk": a_blk,
                "cvec": cvec,
                "wvo_blk": wvo_blk,
                "bo2_rep": bo2_rep,
            }
        )
    return in_maps


def kernel(h_pos, h_geo, n_clusters, Wq, bq, Wk, bk, Wv, bv, Wo, bo, **kwargs):
    assert int(n_clusters) == C_TOTAL
    nc = _get_program()
    in_maps = make_in_maps(h_pos, h_geo, Wq, bq, Wk, bk, Wv, bv, Wo, bo)
    res = run_bass_kernel_spmd(nc, in_maps, core_ids=list(range(N_CORES)))
    shards = [r["out"].reshape(B_LOC, N, D) for r in res.results]
    return np.concatenate(shards, axis=0).astype(np.float32)


# revision 28
# speedup vs baseline: 1.1970x; 1.0074x over previous
"""DLSA block (clustered sparse attention) Trainium2 kernel.

Full-input contract: kernel(**inputs) takes the complete unsharded tensors,
shards batch-dim across 8 NeuronCores, runs a Bass/Tile kernel per core, and
gathers the full output on host.

Host-side marshaling: h_geo/h_pos are uploaded pre-transposed per cluster
([B, C, D, S] layout) so the kernel needs no on-chip transposes and DMA
descriptors are 512B (cluster-feature rows) instead of 128B point rows.

Algebraic folds done on host (weight-space only, float64 for accuracy):
  A    = Wq^T @ Wk / sqrt(D)      -> scores S = Xg A Xg^T + (bq Wk/sqrt(D)) Xg^T
  bk drops entirely (adds a per-row constant to scores; softmax-invariant).
  Wvo  = Wo @ Wv                  -> V' = Xp Wvo^T  (V and O projections fused)
  bo2  = bo + Wo @ bv             (bv commutes through attention since rows of
                                   softmax sum to 1; added to V' pre-attention)

Per cluster (S=128 pts, D=32 feats) on device:
  Z'^T[f,s] = blockdiag(A)^T Xg^T + c   (one matmul per 4-cluster group)
  S^T[t,s]  = Xg Z'^T             (4 row-banded matmuls, one PSUM bank/band)
  P^T       = exp(S^T)            (one ACT op per group)
  V''[t,g]  = Xp blockdiag(Wvo)^T + bo2 (one matmul + one batched bias-add)
  F[s,g]    = P^T.T @ [V''|1]     (ones col yields softmax denom r in col 32)
  out       = F * (1/r)           (batched strided evac into the store tile)
"""

import sys

for _p in ("/opt/trn_rl_repo",):
    if _p not in sys.path:
        sys.path.insert(0, _p)

from contextlib import ExitStack

import numpy as np

import concourse.bass as bass
import concourse.tile as tile
from concourse import bacc, mybir
from concourse.bass_utils import run_bass_kernel_spmd

F32 = mybir.dt.float32

B, N, D = 16, 16384, 32
C_TOTAL, S = 128, 128          # clusters per batch, points per cluster
N_CORES = 8
B_LOC = B // N_CORES           # batches per core
ROWS = B_LOC * N               # data rows per core
TROWS = B_LOC * C_TOTAL * D    # rows of the transposed layout [(b,c,f), s]
SC_CLUSTERS = 32               # clusters per superchunk
SC_ROWS = SC_CLUSTERS * S      # output rows per superchunk
SC_TROWS = SC_CLUSTERS * D     # transposed rows per superchunk
N_SC = ROWS // SC_ROWS         # 8 superchunks per core
G = 4                          # clusters per group
GROUPS_PER_SC = SC_CLUSTERS // G


def _build_program():
    nc = bacc.Bacc("TRN2", target_bir_lowering=False, debug=False)

    hgT = nc.dram_tensor("hgT", [TROWS, S], F32, kind="ExternalInput").ap()
    hpT = nc.dram_tensor("hpT", [TROWS, S], F32, kind="ExternalInput").ap()
    a_blk = nc.dram_tensor("a_blk", [128, 128], F32, kind="ExternalInput").ap()
    cvec = nc.dram_tensor("cvec", [128, 1], F32, kind="ExternalInput").ap()
    wvo_blk = nc.dram_tensor("wvo_blk", [128, 128], F32, kind="ExternalInput").ap()
    bo2_rep = nc.dram_tensor("bo2_rep", [128, G * D], F32, kind="ExternalInput").ap()
    out = nc.dram_tensor("out", [ROWS, D], F32, kind="ExternalOutput").ap()

    with tile.TileContext(nc) as tc, ExitStack() as ctx:
        consts = ctx.enter_context(tc.tile_pool(name="consts", bufs=1))
        io_pool = ctx.enter_context(tc.tile_pool(name="io", bufs=2))
        zsb_pool = ctx.enter_context(tc.tile_pool(name="zsb", bufs=2))
        p_pool = ctx.enter_context(tc.tile_pool(name="p", bufs=2))
        small_pool = ctx.enter_context(tc.tile_pool(name="small", bufs=4))
        v33_pool = ctx.enter_context(tc.tile_pool(name="v33", bufs=1))

        # PSUM: 8 banks. Row-band-concurrent matmuls must land in distinct
        # banks per band (same-partition same-bank concurrent drains from
        # different sub-array row bands wedge the device).
        ps_z = ctx.enter_context(tc.tile_pool(name="ps_z", bufs=1, space="PSUM"))
        ps_work = ctx.enter_context(tc.tile_pool(name="ps_work", bufs=1, space="PSUM"))
        ps_v = ctx.enter_context(tc.tile_pool(name="ps_v", bufs=1, space="PSUM"))
        ps_f = ctx.enter_context(tc.tile_pool(name="ps_f", bufs=2, space="PSUM"))

        # constants
        a_sb = consts.tile([128, 128], F32, tag="a_sb")
        nc.sync.dma_start(a_sb[:], a_blk)
        cvec_sb = consts.tile([128, 1], F32, tag="cvec_sb")
        nc.sync.dma_start(cvec_sb[:], cvec)
        wvo_sb = consts.tile([128, 128], F32, tag="wvo_sb")
        nc.sync.dma_start(wvo_sb[:], wvo_blk)
        bo2_sb = consts.tile([128, G * D], F32, tag="bo2_sb")
        nc.sync.dma_start(bo2_sb[:], bo2_rep)

        # v33 ring: [t, (c,33)] with ones in col 32 of each 33-block
        v33_tiles = []
        for i in range(4):
            t = v33_pool.tile([128, G * 33], F32, tag=f"v33_{i}")
            ones_ap = t[:].rearrange("p (c g) -> p c g", g=33)[:, :, 32:33]
            nc.vector.memset(ones_ap, 1.0)
            v33_tiles.append(t)

        g_global = 0
        for sc in range(N_SC):
            rows = slice(sc * SC_ROWS, (sc + 1) * SC_ROWS)
            trow0 = sc * SC_TROWS
            # hgT/hpT superchunk: [(c4,f)=128, (j, s)] — group j's block-diag
            # transposed inputs land directly in matmul-operand layout.
            # Loads split in half so group 0 can start early.
            hg_sc = io_pool.tile([128, GROUPS_PER_SC * S], F32, tag="hg_sc")
            hp_sc = io_pool.tile([128, GROUPS_PER_SC * S], F32, tag="hp_sc")
            q_j = GROUPS_PER_SC // 4
            for h in range(4):
                r0 = trow0 + h * q_j * 128
                jcols = slice(h * q_j * S, (h + 1) * q_j * S)
                nc.sync.dma_start(
                    hg_sc[:, jcols].rearrange("p (j s) -> p j s", j=q_j),
                    hgT[r0 : r0 + q_j * 128, :].rearrange(
                        "(j r) s -> r j s", j=q_j
                    ),
                )
                nc.sync.dma_start(
                    hp_sc[:, jcols].rearrange("p (j s) -> p j s", j=q_j),
                    hpT[r0 : r0 + q_j * 128, :].rearrange(
                        "(j r) s -> r j s", j=q_j
                    ),
                )
            out_sc = io_pool.tile([128, SC_CLUSTERS * D], F32, tag="out_sc")

            for j in range(GROUPS_PER_SC):
                cols = slice(j * G * D, (j + 1) * G * D)
                xg = hg_sc[:, j * S : (j + 1) * S]
                xp = hp_sc[:, j * S : (j + 1) * S]

                # Z'^T[(c,f),s] = blockdiag(A)^T Xg^T (+c at evac)
                z_ps = ps_z.tile([128, 128], F32, tag="z_ps")
                nc.tensor.matmul(z_ps[:], a_sb[:], xg)
                z_sb = zsb_pool.tile([128, 128], F32, tag="z_sb")
                nc.scalar.activation(
                    z_sb[:], z_ps[:], mybir.ActivationFunctionType.Identity,
                    bias=cvec_sb[:],
                )

                # S^T[t,s] = Xg Z'^T: 4 row-banded matmuls, one bank per band
                wk = ps_work.tile([128, 2048], F32, tag="wk")
                for c in range(G):
                    p0 = c * 32
                    nc.tensor.matmul(
                        wk[:, c * 512 : c * 512 + 128],
                        xg[p0 : p0 + 32, :],
                        z_sb[p0 : p0 + 32, :],
                        tile_position=(p0, 0),
                    )
                wk_view = wk[:].rearrange("p (c q) -> p c q", q=512)
                p_sb = p_pool.tile([128, 512], F32, tag="p_sb")
                nc.scalar.activation(
                    p_sb[:].rearrange("p (c q) -> p c q", q=128),
                    wk_view[:, :, 0:128],
                    mybir.ActivationFunctionType.Exp,
                )

                # V'[t,(c,g)] = Xp blockdiag(Wvo^T): one matmul
                v_ps = ps_v.tile([128, 128], F32, tag="v_ps")
                nc.tensor.matmul(v_ps[:], xp, wvo_sb[:])
                # V'' = V' + bo2, strided into the v33 ring (ones col kept)
                v33 = v33_tiles[g_global % 4]
                nc.vector.tensor_tensor(
                    v33[:].rearrange("p (c g) -> p c g", g=33)[:, :, 0:32],
                    v_ps[:].rearrange("p (c g) -> p c g", g=D),
                    bo2_sb[:].rearrange("p (c g) -> p c g", g=D),
                    mybir.AluOpType.add,
                )

                # F_un[s,(c,33)] = P^T.T @ [V''|1]; col 32 of block = r[s]
                f_ps = ps_f.tile([128, G * 33], F32, tag="f_ps")
                for c in range(G):
                    nc.tensor.matmul(
                        f_ps[:, c * 33 : (c + 1) * 33],
                        p_sb[:, c * 128 : (c + 1) * 128],
                        v33[:, c * 33 : (c + 1) * 33],
                        tile_position=(0, 0),
                    )
                f_view = f_ps[:].rearrange("p (c g) -> p c g", g=33)
                recip = small_pool.tile([128, G], F32, tag="recip")
                nc.vector.reciprocal(recip[:, :, None], f_view[:, :, 32:33])
                nc.vector.tensor_tensor(
                    out_sc[:, cols].rearrange("p (c d) -> p c d", d=D),
                    f_view[:, :, 0:32],
                    recip[:, :, None].to_broadcast([128, G, D]),
                    mybir.AluOpType.mult,
                )
                g_global += 1

            # store in halves so the first half drains while the second half
            # of the superchunk is still computing
            hc = SC_CLUSTERS // 2
            for h in range(2):
                hrows = slice(
                    sc * SC_ROWS + h * hc * S, sc * SC_ROWS + (h + 1) * hc * S
                )
                hcols = slice(h * hc * D, (h + 1) * hc * D)
                nc.sync.dma_start(
                    out[hrows, :].rearrange("(c s) d -> s c d", s=S),
                    out_sc[:, hcols].rearrange("p (c d) -> p c d", d=D),
                )

    nc.compile()
    return nc


_PROGRAM = None


def _get_program():
    global _PROGRAM
    if _PROGRAM is None:
        _PROGRAM = _build_program()
    return _PROGRAM


def _host_fold(Wq, bq, Wk, bk, Wv, bv, Wo, bo):
    Wq64, Wk64 = np.asarray(Wq, np.float64), np.asarray(Wk, np.float64)
    Wv64, Wo64 = np.asarray(Wv, np.float64), np.asarray(Wo, np.float64)
    bq64, bv64, bo64 = (np.asarray(x, np.float64) for x in (bq, bv, bo))
    scale = 1.0 / np.sqrt(np.float64(D))
    A = (Wq64.T @ Wk64) * scale                      # [e, f]
    c = (bq64 @ Wk64) * scale                        # [f]
    WvoT = (Wo64 @ Wv64).T                           # [e, g]
    bo2 = bo64 + Wo64 @ bv64                         # [g]
    a_blk = np.zeros((128, 128), np.float32)
    wvo_blk = np.zeros((128, 128), np.float32)
    for cc in range(G):
        a_blk[cc * D : (cc + 1) * D, cc * D : (cc + 1) * D] = A
        wvo_blk[cc * D : (cc + 1) * D, cc * D : (cc + 1) * D] = WvoT
    cvec = np.tile(c, G)[:, None].astype(np.float32)         # [128, 1]
    bo2_rep = np.tile(bo2, (128, G)).reshape(128, G * D).astype(np.float32)
    return a_blk, cvec, wvo_blk, bo2_rep


def make_in_maps(h_pos, h_geo, Wq, bq, Wk, bk, Wv, bv, Wo, bo):
    a_blk, cvec, wvo_blk, bo2_rep = _host_fold(Wq, bq, Wk, bk, Wv, bv, Wo, bo)
    # per-cluster transpose on host: [B, N, D] -> [B, C, D, S]
    hgT_full = np.ascontiguousarray(
        np.asarray(h_geo, np.float32).reshape(B, C_TOTAL, S, D).transpose(0, 1, 3, 2)
    ).reshape(B * C_TOTAL * D, S)
    hpT_full = np.ascontiguousarray(
        np.asarray(h_pos, np.float32).reshape(B, C_TOTAL, S, D).transpose(0, 1, 3, 2)
    ).reshape(B * C_TOTAL * D, S)
    in_maps = []
    for core in range(N_CORES):
        trows = slice(core * TROWS, (core + 1) * TROWS)
        in_maps.append(
            {
                "hgT": np.ascontiguousarray(hgT_full[trows]),
                "hpT": np.ascontiguousarray(hpT_full[trows]),
                "a_blk": a_blk,
                "cvec": cvec,
                "wvo_blk": wvo_blk,
                "bo2_rep": bo2_rep,
            }
        )
    return in_maps


def kernel(h_pos, h_geo, n_clusters, Wq, bq, Wk, bk, Wv, bv, Wo, bo, **kwargs):
    assert int(n_clusters) == C_TOTAL
    nc = _get_program()
    in_maps = make_in_maps(h_pos, h_geo, Wq, bq, Wk, bk, Wv, bv, Wo, bo)
    res = run_bass_kernel_spmd(nc, in_maps, core_ids=list(range(N_CORES)))
    shards = [r["out"].reshape(B_LOC, N, D) for r in res.results]
    return np.concatenate(shards, axis=0).astype(np.float32)
